# revision 8
# baseline (speedup 1.0000x reference)
"""MixtureOfLinear (base linear + top-2-of-8 LoRA experts) on 8 trn2 cores.

Sharding: pure data-parallel over tokens. T = 4*2048 = 8192 tokens split into
8 shards of 1024. Every core holds the full (bf16) W_base^T, the LoRA factors
and the router; no collectives.

Host-side prep (not on HW): cast to bf16 and pre-transpose so that the
contraction dim (d_in) lands on SBUF partitions for both matmul operands:
  xt   = x_shard^T          [4096, 1024]  bf16
  wt   = W_base^T           [4096, 4096]  bf16
  rt   = [A_flat^T | W_router^T] [4096, 144] bf16 (h and router logits in one
         matmul stream; 136 used, padded to 144)
  bmat = 2.0 * B[e,o,r] -> [e*16+r, o]  [128, 4096] bf16 (LoRA scaling folded)
  bias = b_base             [4096] f32

Device per core:
  phase 0: h/logits = x @ [A|Wr]^T; top-2 softmax weights via max/mask/exp on
           DVE/ACT; hw = h * w (broadcast over rank); PE-transpose hw to
           hwT [er, t] per 128-token tile.
  phase 1: for each 512-wide d_out chunk: 32 K-tiles of base matmul + 1 LoRA
           matmul (lhsT=hwT, rhs=bmat chunk) accumulated in the same PSUM
           tile; add bias; DMA out fp32.
"""

import json
import os

import numpy as np
import ml_dtypes

import concourse.bass as bass
import concourse.tile as tile
from concourse import bass2jax, mybir
from concourse.bass import ts
from concourse.bass_utils import run_bass_kernel_spmd
from concourse.masks import make_identity


def _split_waits_in_bir(bir_json: bytes) -> bytes:
    """Hoist multi-wait sync_info onto standalone EventSemaphore instructions.

    The walrus build here rejects instructions carrying >1 on_wait ("Too many
    sync wait commands"), while Tile's sem assignment freely attaches 2-3.
    Inserting same-engine EventSemaphore instructions immediately before the
    offender is semantically equivalent (the queue blocks in order)."""
    bir = json.loads(bir_json)
    counter = [0]

    def fix_block(block):
        out = []
        for inst in block.get("instructions", []):
            si = inst.get("sync_info")
            ow = (si or {}).get("on_wait") or []
            if si is not None and len(ow) > 1 and inst.get("opcode") != "EventSemaphore":
                for w in ow[:-1]:
                    counter[0] += 1
                    out.append(
                        {
                            "debug": inst.get("debug", 0),
                            "engine": inst["engine"],
                            "ins": [],
                            "name": f"syncsplit_{counter[0]}_{inst['name']}",
                            "opcode": "EventSemaphore",
                            "outs": [],
                            "sync_info": {"on_update": [], "on_wait": [w]},
                        }
                    )
                si["on_wait"] = ow[-1:]
            out.append(inst)
        block["instructions"] = out
        for b in block.get("blocks", []):
            fix_block(b)

    for fn in bir.get("functions", []):
        for b in fn.get("blocks", []):
            fix_block(b)
    return json.dumps(bir).encode()


if not getattr(bass2jax, "_syncsplit_patched", False):
    _orig_compile_bir_kernel = bass2jax.compile_bir_kernel

    def _compile_with_split(bir_json, tmpdir, neff_name="file.neff"):
        return _orig_compile_bir_kernel(
            _split_waits_in_bir(bir_json), tmpdir, neff_name=neff_name
        )

    bass2jax.compile_bir_kernel = _compile_with_split
    bass2jax._syncsplit_patched = True

BF16 = ml_dtypes.bfloat16

N_CORES = 8
B_DIM, S_DIM, D_IN, D_OUT = 4, 2048, 4096, 4096
T_FULL = B_DIM * S_DIM          # 8192
T_LOC = T_FULL // N_CORES       # 1024
NUM_EXPERTS = 8
RANK = 16
SCALING = 2.0
ER = NUM_EXPERTS * RANK         # 128
RW = ER + NUM_EXPERTS           # 136 (h | logits)
RW_PAD = 144                    # pad free dim a bit

P = 128
KT = D_IN // P                  # 32 k-tiles
NT = T_LOC // P                 # 8 token tiles
OCH = 512                       # d_out chunk
NO = D_OUT // OCH               # 8 chunks

# results of the last run (for test.py to read exec_time_ns etc.)
LAST_RESULTS = None


def _ensure_ntff_hook():
    """The agent image's ``antenv`` lacks ``axon_hooks``, so boot() skipped
    registering the NTFF profile hook. Synthesize the module and register the
    ctypes-based hook so ``trace=True`` yields HW exec times."""
    import sys
    import types

    try:
        from antenv.axon_hooks import get_axon_ntff_profile_hook  # noqa: F401
        return True
    except ImportError:
        pass
    try:
        import antenv
        from trn_agent_boot.trn_boot import _ntff_profile_via_ctypes

        mod = types.ModuleType("antenv.axon_hooks")
        mod._hook = None

        def set_axon_ntff_profile_hook(h):
            mod._hook = h

        def get_axon_ntff_profile_hook():
            return mod._hook

        mod.set_axon_ntff_profile_hook = set_axon_ntff_profile_hook
        mod.get_axon_ntff_profile_hook = get_axon_ntff_profile_hook
        sys.modules["antenv.axon_hooks"] = mod
        antenv.axon_hooks = mod
        mod._hook = _ntff_profile_via_ctypes("/opt/axon/libaxon_pjrt.so")
        return mod._hook is not None
    except Exception:
        return False


def _build_bass():
    nc = bass.Bass()
    f32 = mybir.dt.float32
    bf16 = mybir.dt.bfloat16

    xt = nc.dram_tensor("xt", [D_IN, T_LOC], bf16, kind="ExternalInput")
    wt = nc.dram_tensor("wt", [D_IN, D_OUT], bf16, kind="ExternalInput")
    rt = nc.dram_tensor("rt", [D_IN, RW_PAD], bf16, kind="ExternalInput")
    bmat = nc.dram_tensor("bmat", [ER, D_OUT], bf16, kind="ExternalInput")
    bias = nc.dram_tensor("bias", [D_OUT], f32, kind="ExternalInput")
    out = nc.dram_tensor("out", [T_LOC, D_OUT], f32, kind="ExternalOutput")

    xt_r = xt[:].rearrange("(ko p) t -> p ko t", p=P)      # [128, 32, 1024]
    wt_r = wt[:].rearrange("(ko p) o -> p ko o", p=P)      # [128, 32, 4096]
    rt_r = rt[:].rearrange("(ko p) n -> p ko n", p=P)      # [128, 32, 144]

    with tile.TileContext(nc) as tc:
        with (
            tc.tile_pool(name="singles", bufs=1) as singles,
            tc.tile_pool(name="wpool", bufs=2) as wpool,
            tc.tile_pool(name="hpool", bufs=2) as hpool,
            tc.tile_pool(name="opool", bufs=3) as opool,
            tc.tile_pool(name="pmain", bufs=4, space="PSUM") as pmain,
            tc.tile_pool(name="ph", bufs=2, space="PSUM") as ph,
            tc.tile_pool(name="ptr", bufs=2, space="PSUM") as ptr,
        ):
            f32_ = mybir.dt.float32
            bf16_ = mybir.dt.bfloat16

            # resident inputs
            xt_sb = singles.tile([P, KT, T_LOC], bf16_)
            nc.sync.dma_start(xt_sb[:], xt_r)
            rt_sb = singles.tile([P, KT, RW_PAD], bf16_)
            nc.sync.dma_start(rt_sb[:], rt_r)
            bmat_sb = singles.tile([P, D_OUT], bf16_)
            nc.sync.dma_start(bmat_sb[:], bmat[:])
            bias_rep = singles.tile([P, D_OUT], f32_)
            bias_ap = bias[:]
            bias_bcast = bass.AP(
                tensor=bias_ap.tensor,
                offset=bias_ap.offset,
                ap=[[0, P]] + list(bias_ap.ap),
            )
            nc.gpsimd.dma_start(bias_rep[:], bias_bcast)
            ident = singles.tile([P, P], bf16_)
            make_identity(nc, ident)
            hwT_sb = singles.tile([P, NT, P], bf16_)

            E = NUM_EXPERTS

            # ---- phase 0: router + LoRA down-projection -------------------
            # h/logits for all NT token tiles, then the whole top-2-softmax
            # as a handful of batched DVE ops (a single Exp on ACT).
            h_all = singles.tile([P, NT, ER], f32_)
            lg_all = singles.tile([P, NT, E], f32_)
            for tt in range(NT):
                ph_t = ph.tile([P, RW], f32_)
                for k in range(KT):
                    nc.tensor.matmul(
                        ph_t[:],
                        lhsT=xt_sb[:, k, ts(tt, P)],
                        rhs=rt_sb[:, k, :RW],
                        start=(k == 0),
                        stop=(k == KT - 1),
                    )
                nc.vector.tensor_copy(h_all[:, tt, :], ph_t[:, :ER])
                nc.vector.tensor_copy(lg_all[:, tt, :], ph_t[:, ER:RW])

            m1 = hpool.tile([P, NT, 1], f32_)
            nc.vector.reduce_max(m1[:], lg_all[:], axis=mybir.AxisListType.X)
            eq1 = hpool.tile([P, NT, E], f32_)
            nc.vector.tensor_tensor(
                eq1[:], lg_all[:], m1[:].to_broadcast((P, NT, E)),
                mybir.AluOpType.is_equal,
            )
            lg2 = hpool.tile([P, NT, E], f32_)
            nc.vector.tensor_scalar_mul(lg2[:], eq1[:], -1.0e30)
            nc.vector.tensor_add(lg2[:], lg2[:], lg_all[:])
            m2 = hpool.tile([P, NT, 1], f32_)
            nc.vector.reduce_max(m2[:], lg2[:], axis=mybir.AxisListType.X)
            eq2 = hpool.tile([P, NT, E], f32_)
            nc.vector.tensor_tensor(
                eq2[:], lg2[:], m2[:].to_broadcast((P, NT, E)),
                mybir.AluOpType.is_equal,
            )
            d21 = hpool.tile([P, NT], f32_)
            nc.vector.tensor_sub(
                d21[:], m2[:, :, 0], m1[:, :, 0]
            )
            e2 = hpool.tile([P, NT], f32_)
            nc.scalar.activation(
                e2[:], d21[:], mybir.ActivationFunctionType.Exp
            )
            den = hpool.tile([P, NT], f32_)
            nc.vector.tensor_scalar_add(den[:], e2[:], 1.0)
            w1 = hpool.tile([P, NT], f32_)
            nc.vector.reciprocal(w1[:], den[:])
            w2 = hpool.tile([P, NT], f32_)
            nc.vector.tensor_mul(w2[:], e2[:], w1[:])
            we = hpool.tile([P, NT, E], f32_)
            nc.vector.tensor_tensor(
                we[:], eq1[:], w1[:, :, None].to_broadcast((P, NT, E)),
                mybir.AluOpType.mult,
            )
            we2 = hpool.tile([P, NT, E], f32_)
            nc.vector.tensor_tensor(
                we2[:], eq2[:], w2[:, :, None].to_broadcast((P, NT, E)),
                mybir.AluOpType.mult,
            )
            nc.vector.tensor_add(we[:], we[:], we2[:])
            # hw[t, nt, e, r] = h[t, nt, e, r] * we[t, nt, e] (cast to bf16)
            hw_all = singles.tile([P, NT, ER], bf16_)
            nc.vector.tensor_tensor(
                hw_all[:].rearrange("p n (e r) -> p n e r", r=RANK),
                h_all[:].rearrange("p n (e r) -> p n e r", r=RANK),
                we[:, :, :, None].to_broadcast((P, NT, E, RANK)),
                mybir.AluOpType.mult,
            )
            # transpose each [t, er] tile to [er, t] for the up-projection
            for tt in range(NT):
                ptr_t = ptr.tile([P, P], bf16_)
                nc.tensor.transpose(ptr_t[:], hw_all[:, tt, :], ident[:])
                nc.vector.tensor_copy(hwT_sb[:, tt, :], ptr_t[:])

            # ---- phase 1: base matmul + LoRA up-projection ----------------
            for oc in range(NO):
                wt_sb = wpool.tile([P, KT, OCH], bf16_)
                nc.sync.dma_start(wt_sb[:], wt_r[:, :, ts(oc, OCH)])
                for tt in range(NT):
                    ps = pmain.tile([P, OCH], f32_)
                    for k in range(KT):
                        nc.tensor.matmul(
                            ps[:],
                            lhsT=xt_sb[:, k, ts(tt, P)],
                            rhs=wt_sb[:, k, :],
                            start=(k == 0),
                            stop=False,
                        )
                    nc.tensor.matmul(
                        ps[:],
                        lhsT=hwT_sb[:, tt, :],
                        rhs=bmat_sb[:, ts(oc, OCH)],
                        start=False,
                        stop=True,
                    )
                    ot = opool.tile([P, OCH], f32_)
                    nc.vector.tensor_add(ot[:], ps[:], bias_rep[:, ts(oc, OCH)])
                    nc.sync.dma_start(out[ts(tt, P), ts(oc, OCH)], ot[:])

    return nc


def kernel(x, W_base, b_base, W_router, A, B):
    global LAST_RESULTS

    xf = np.asarray(x, dtype=np.float32).reshape(T_FULL, D_IN)

    wt_np = np.ascontiguousarray(np.asarray(W_base, np.float32).T).astype(BF16)
    a_flat = np.asarray(A, np.float32).reshape(ER, D_IN)          # er = e*16+r
    rt_np = np.zeros((D_IN, RW_PAD), dtype=BF16)
    rt_np[:, :ER] = a_flat.T.astype(BF16)
    rt_np[:, ER:RW] = np.asarray(W_router, np.float32).T.astype(BF16)
    bmat_np = np.ascontiguousarray(
        np.asarray(B, np.float32).transpose(0, 2, 1).reshape(ER, D_OUT) * SCALING
    ).astype(BF16)
    bias_np = np.ascontiguousarray(np.asarray(b_base, np.float32))

    in_maps = []
    for c in range(N_CORES):
        xt_np = np.ascontiguousarray(
            xf[c * T_LOC : (c + 1) * T_LOC].T
        ).astype(BF16)
        in_maps.append(
            {
                "xt": xt_np,
                "wt": wt_np,
                "rt": rt_np,
                "bmat": bmat_np,
                "bias": bias_np,
            }
        )

    nc = _build_bass()
    trace = os.environ.get("KERNEL_TRACE", "0") == "1"
    if trace:
        trace = _ensure_ntff_hook()
    res = run_bass_kernel_spmd(
        nc, in_maps, core_ids=list(range(N_CORES)), trace=trace
    )
    LAST_RESULTS = res

    out = np.concatenate(
        [res.results[c]["out"] for c in range(N_CORES)], axis=0
    )
    return out.reshape(B_DIM, S_DIM, D_OUT)


if __name__ == "__main__":
    rng = np.random.default_rng(0)
    x = rng.standard_normal((B_DIM, S_DIM, D_IN), dtype=np.float32)
    W = rng.standard_normal((D_OUT, D_IN), dtype=np.float32) * 0.02
    b = rng.standard_normal((D_OUT,), dtype=np.float32) * 0.02
    Wr = rng.standard_normal((NUM_EXPERTS, D_IN), dtype=np.float32) * 0.02
    A = rng.standard_normal((NUM_EXPERTS, RANK, D_IN), dtype=np.float32) * 0.02
    Bm = rng.standard_normal((NUM_EXPERTS, D_OUT, RANK), dtype=np.float32) * 0.02
    o = kernel(x, W, b, Wr, A, Bm)
    print(o.shape, o.dtype, float(np.abs(o).mean()))
